# revision 42
# baseline (speedup 1.0000x reference)
"""Trainium2 Bass kernel for BaseLinearLayerWithLoRA (moe_routing).

out = x @ W^T + b  +  per-token LoRA:  out[t] += (x[t] @ A[l]^T) @ B[l]^T,  l = idx[t]

Sharding: data-parallel over tokens across 8 NeuronCores (4096 tokens each);
W, bias and the stacked LoRA A/B tables are replicated.

Per-core kernel design (single pass over tokens, all-bf16 operands with fp32
PSUM accumulation, ~2e-3 rms error vs the fp32 reference):
  - x is host-retiled to [super-block, partition, c-chunk, token] so each
    512-token super-block is one 2 MB line-rate DMA (8 KB contiguous per
    partition); the naive x^T layout would pay the <512 B read-mod-write tax.
  - Base GEMM: stationary = x^T chunk [128 d_in x 128 tokens], moving =
    W^T chunk [128, 512]; 4-wide o-sweep per stationary into 4 PSUM banks.
  - The LoRA expand is fused into the base GEMM as a 17th contraction
    chunk: wt chunk 16 holds the stacked B table [128 ranks, D_OUT], and
    xt chunk 16 holds the masked shrink S_m [128 ranks, tokens].
  - LoRA shrink for super-block s+1 runs during super-block s, its 16 MMs
    interleaved 1-per-4 into block 1's c-loop (a 16-MM same-PSUM-bank burst
    measures ~2x roofline; interleaving hides the stationary reloads).  The
    host-precomputed one-hot mask (mask[r,t] = r//16==idx[t]) zeroes foreign
    adapters' rows via one DVE multiply straight into xt chunk 16.
  - Bias is added during the PSUM->SBUF drain (DVE, host-replicated to 128
    rows); each block stores one full-row 1 MB tile.
  - Ring split: x stream + at on the SP HWDGE ring; W loads and out stores
    on the ACT ring; mask/B-table/bias on SWDGE — out stores never
    head-of-line-block the latency-critical x stream.
"""

import contextlib
import sys

for _p in ("/opt/trn_rl_repo", "/root/.axon_site/_ro/trn_rl_repo"):
    if _p not in sys.path:
        sys.path.insert(0, _p)

import numpy as np
import ml_dtypes

import concourse.bass as bass  # noqa: F401  (registers engines)
import concourse.mybir as mybir
import concourse.tile as tile
from concourse import bacc
from concourse.bass_utils import run_bass_kernel_spmd

N_CORES = 8
T_FULL, D_IN, D_OUT = 32768, 2048, 2048
MAX_LORAS, RANK = 8, 16
T_CORE = T_FULL // N_CORES          # 4096 tokens per core
SB_T = 512                          # super-block tokens
N_SB = T_CORE // SB_T               # 8 super-blocks
N_BLK = SB_T // 128                 # 4 token blocks per super-block
KC = D_IN // 128                    # 16 contraction chunks
N_OT = D_OUT // 512                 # 4 o-tiles (full width resident)

_CACHED = {}


def _build(reps=1, lora=True, store=True, xdma=True, shrink1=False):
    # reps>1 wraps the whole body in a device-side For_i loop (same static
    # addresses each iteration) — used only by the timing harness to amortize
    # launch overhead; the graded kernel path uses reps=1.  lora/store/xdma
    # are ablation switches for HW bottleneck attribution (timing harness
    # only — they break correctness).
    key = ("nc", reps, lora, store, xdma, shrink1)
    if key in _CACHED:
        return _CACHED[key]
    kc_eff = KC + 1 if lora else KC  # chunk 16 = fused LoRA expand

    f32 = mybir.dt.float32
    bf16 = mybir.dt.bfloat16

    nc = bacc.Bacc("TRN2", target_bir_lowering=False, debug=False)

    # xP[s, p, c*SB_T + t] = x[s*SB_T + t, c*128 + p]: one contiguous 8 KB run
    # per (super-block, partition) so each super-block is a single 1 MB DMA at
    # line rate (256 B runs of the naive x^T layout pay the <512 B RMW tax).
    xP = nc.dram_tensor("xP", [N_SB * 128, KC * SB_T], bf16, kind="ExternalInput")
    wT = nc.dram_tensor("wT", [D_IN, D_OUT], bf16, kind="ExternalInput")
    aT = nc.dram_tensor("aT", [D_IN, 128], bf16, kind="ExternalInput")
    bT = nc.dram_tensor("bT", [128, D_OUT], bf16, kind="ExternalInput")
    maskM = nc.dram_tensor("maskM", [128, T_CORE], bf16, kind="ExternalInput")
    bias_rep = nc.dram_tensor("bias_rep", [128, D_OUT], f32, kind="ExternalInput")
    out = nc.dram_tensor("out", [T_CORE, D_OUT], f32, kind="ExternalOutput")

    xP_v = xP.rearrange("(s p) q -> p s q", p=128)      # [128, N_SB, KC*SB_T]
    wT_v = wT.rearrange("(c p) o -> p c o", p=128)      # [128, 16, 2048]
    aT_v = aT.rearrange("(c p) r -> p c r", p=128)      # [128, 16, 128]

    with tile.TileContext(nc) as tc:
        with (
            tc.tile_pool(name="const", bufs=1) as const,
            tc.tile_pool(name="wpool", bufs=1) as wpool,
            tc.tile_pool(name="xpool", bufs=3) as xpool,
            tc.tile_pool(name="opool", bufs=3) as opool,
            tc.tile_pool(name="pso", bufs=8, space="PSUM") as pso,
        ):
            at = const.tile([128, KC, 128], bf16)
            bias_t = const.tile([128, D_OUT], bf16)
            mall = const.tile([128, T_CORE], bf16)
            # chunk KC of wt holds the stacked LoRA B table: the expand is
            # just the 17th contraction chunk of the base GEMM.
            wt = wpool.tile([128, KC + 1, D_OUT], bf16)

            rep_cm = tc.For_i(0, reps) if reps > 1 else contextlib.nullcontext()
            with rep_cm:
                def load_x(s, split=1):
                    # chunk KC is filled by shrink()'s DVE mask-multiply.
                    # split>1 loads in c-chunk groups so SB0's prologue
                    # shrink can start after the first group lands.
                    xt = xpool.tile([128, KC + 1, SB_T], bf16, tag="xt",
                                    name=f"xt{s}")
                    gs = KC // split
                    for g in range(split):
                        nc.sync.dma_start(
                            xt[:, g * gs:(g + 1) * gs, :]
                            .rearrange("p c t -> p (c t)"),
                            xP_v[:, s, g * gs * SB_T:(g + 1) * gs * SB_T])
                    return xt

                xts = [None] * N_SB
                # SP-ring order: at (small, gates first shrink LDW), then
                # SB0's x in 2 groups (first shrink MM after group 0 lands);
                # mask/B-table/bias ride SWDGE — not needed for ~10 us.
                nc.sync.dma_start(at[:], aT_v[:])
                xts[0] = load_x(0, split=2)
                for c in range(KC):
                    # W loads ride the second HWDGE ring (ACT) so they don't
                    # head-of-line-block the x stream on the SP ring.
                    nc.scalar.dma_start(wt[:, c, :], wT_v[:, c, :])
                nc.gpsimd.dma_start(mall[:], maskM[:])
                nc.gpsimd.dma_start(wt[:, KC, :], bT[:])
                nc.gpsimd.dma_start(bias_t[:], bias_rep[:])  # SWDGE cast

                n_shr = 1 if shrink1 else KC

                def shrink_mm(ps_s, s, xt_s, c):
                    nc.tensor.matmul(ps_s[:, :SB_T], at[:, c, :],
                                     xt_s[:, c, :],
                                     start=(c == 0), stop=(c == n_shr - 1))

                def shrink_fin(ps_s, s, xt_s):
                    # mask zeroes foreign adapters per token column; result
                    # lands in xt chunk KC = the expand's lhsT.
                    nc.vector.tensor_tensor(
                        xt_s[:, KC, :], ps_s[:, :SB_T],
                        mall[:, s * SB_T:(s + 1) * SB_T],
                        mybir.AluOpType.mult)

                def shrink(s, xt_s):
                    # standalone masked LoRA shrink (SB0 prologue only)
                    ps_s = pso.tile([128, 512], f32, tag="ps_o", name="ps_s")
                    for c in range(n_shr):
                        shrink_mm(ps_s, s, xt_s, c)
                    shrink_fin(ps_s, s, xt_s)
                for s in range(N_SB):
                    t0 = s * SB_T
                    if xdma and s + 1 < N_SB:
                        xts[s + 1] = load_x(s + 1)
                    xt = xts[s]
                    if lora and s == 0:
                        # SB0's own shrink, ahead of its block loop: only
                        # block 0's final (c=16) matmul can briefly wait on
                        # the DVE mask-multiply.
                        shrink(0, xt)
                    for b in range(N_BLK):
                        tb = b * 128
                        # next super-block's shrink MMs interleave into block
                        # 1's c-loop (1 shrink MM per 4 base MMs): stationary
                        # reloads hide behind the base group and consecutive
                        # shrink MMs never hit the same PSUM bank
                        # back-to-back — a 16-MM same-bank burst measures
                        # ~2x its roofline cost.
                        inter_shrink = lora and b == 1 and s + 1 < N_SB
                        if inter_shrink:
                            xt_n = xts[s + 1] if xdma else xt
                            ps_s = pso.tile([128, 512], f32, tag="ps_o",
                                            name="ps_s")
                        psums = [
                            pso.tile([128, 512], f32, tag="ps_o",
                                     name=f"ps_o{o}")
                            for o in range(N_OT)
                        ]
                        for c in range(kc_eff):
                            for o in range(N_OT):
                                nc.tensor.matmul(
                                    psums[o][:],
                                    xt[:, c, tb:tb + 128],
                                    wt[:, c, o * 512:(o + 1) * 512],
                                    start=(c == 0),
                                    stop=(c == kc_eff - 1))
                            if inter_shrink and c < n_shr:
                                shrink_mm(ps_s, s + 1, xt_n, c)
                        if inter_shrink:
                            shrink_fin(ps_s, s + 1, xt_n)
                        if not store:
                            continue
                        ot = opool.tile([128, D_OUT], f32, tag="ot", name="ot")
                        for o in range(N_OT):
                            nc.vector.tensor_tensor(
                                ot[:, o * 512:(o + 1) * 512], psums[o][:],
                                bias_t[:, o * 512:(o + 1) * 512],
                                mybir.AluOpType.add)
                        # one full-row 1 MB store per block (8 KB/partition
                        # contiguous, line-rate) on the ACT ring, which is
                        # idle once W is loaded — never blocks the x stream.
                        nc.scalar.dma_start(out[t0 + tb:t0 + tb + 128, :],
                                            ot[:])

    nc.compile()
    _CACHED[key] = nc
    return nc


def _prep_inputs(x, base_weight, base_bias, lora_a, lora_b, token_lora_indices):
    bf16 = ml_dtypes.bfloat16
    x = np.asarray(x, dtype=np.float32)
    w = np.asarray(base_weight, dtype=np.float32)
    bias = np.asarray(base_bias, dtype=np.float32)
    la = np.asarray(lora_a, dtype=np.float32)
    lb = np.asarray(lora_b, dtype=np.float32)
    idx = np.asarray(token_lora_indices, dtype=np.int32)

    wT = np.ascontiguousarray(w.T).astype(bf16)                      # [D_IN, D_OUT]
    aT = np.ascontiguousarray(la.reshape(128, D_IN).T).astype(bf16)  # [D_IN, 128]
    bT = np.ascontiguousarray(
        lb[:, 0].transpose(0, 2, 1).reshape(128, D_OUT)).astype(bf16)
    bias_rep = np.ascontiguousarray(
        np.broadcast_to(bias[None, :], (128, D_OUT)))                # [128, D_OUT]
    mask = (np.arange(128, dtype=np.int32)[:, None] // RANK
            == idx[None, :]).astype(bf16)                            # [128, T_FULL]

    # xP[s, p, c, t] = x[s*SB_T + t, c*128 + p] per core: one contiguous 8 KB
    # run per (super-block, partition) -> line-rate DMA.
    xP = (x.reshape(N_CORES, N_SB, SB_T, KC, 128)
          .transpose(0, 1, 4, 3, 2)
          .reshape(N_CORES, N_SB * 128, KC * SB_T).astype(bf16))
    in_maps = []
    for c in range(N_CORES):
        sl = slice(c * T_CORE, (c + 1) * T_CORE)
        in_maps.append({
            "xP": np.ascontiguousarray(xP[c]),
            "wT": wT,
            "aT": aT,
            "bT": bT,
            "maskM": np.ascontiguousarray(mask[:, sl]),
            "bias_rep": bias_rep,
        })
    return in_maps


def kernel(x, base_weight, base_bias, lora_a, lora_b, token_lora_indices):
    nc = _build()
    in_maps = _prep_inputs(x, base_weight, base_bias, lora_a, lora_b,
                           token_lora_indices)
    res = run_bass_kernel_spmd(nc, in_maps, list(range(N_CORES)))
    return np.concatenate([res.results[c]["out"] for c in range(N_CORES)], axis=0)



# revision 44
# speedup vs baseline: 1.0292x; 1.0292x over previous
"""Trainium2 Bass kernel for BaseLinearLayerWithLoRA (moe_routing).

out = x @ W^T + b  +  per-token LoRA:  out[t] += (x[t] @ A[l]^T) @ B[l]^T,  l = idx[t]

Sharding: data-parallel over tokens across 8 NeuronCores (4096 tokens each);
W, bias and the stacked LoRA A/B tables are replicated.

Per-core kernel design (single pass over tokens, all-bf16 operands with fp32
PSUM accumulation, ~2e-3 rms error vs the fp32 reference):
  - x is host-retiled to [super-block, partition, c-chunk, token] so each
    512-token super-block is one 2 MB line-rate DMA (8 KB contiguous per
    partition); the naive x^T layout would pay the <512 B read-mod-write tax.
  - Base GEMM: stationary = x^T chunk [128 d_in x 128 tokens], moving =
    W^T chunk [128, 512]; 4-wide o-sweep per stationary into 4 PSUM banks.
  - The LoRA expand is fused into the base GEMM as a 17th contraction
    chunk: wt chunk 16 holds the stacked B table [128 ranks, D_OUT], and
    xt chunk 16 holds the masked shrink S_m [128 ranks, tokens].
  - LoRA shrink for super-block s+1 runs during super-block s, its 16 MMs
    interleaved 1-per-4 into block 1's c-loop (a 16-MM same-PSUM-bank burst
    measures ~2x roofline; interleaving hides the stationary reloads).  The
    host-precomputed one-hot mask (mask[r,t] = r//16==idx[t]) zeroes foreign
    adapters' rows via one DVE multiply straight into xt chunk 16.
  - Bias is added during the PSUM->SBUF drain (DVE, host-replicated to 128
    rows); each block stores one full-row 1 MB tile.
  - Ring split: x stream + at on the SP HWDGE ring; W loads and out stores
    on the ACT ring; mask/B-table/bias on SWDGE — out stores never
    head-of-line-block the latency-critical x stream.
"""

import contextlib
import sys

for _p in ("/opt/trn_rl_repo", "/root/.axon_site/_ro/trn_rl_repo"):
    if _p not in sys.path:
        sys.path.insert(0, _p)

import numpy as np
import ml_dtypes

import concourse.bass as bass  # noqa: F401  (registers engines)
import concourse.mybir as mybir
import concourse.tile as tile
from concourse import bacc
from concourse.bass_utils import run_bass_kernel_spmd

N_CORES = 8
T_FULL, D_IN, D_OUT = 32768, 2048, 2048
MAX_LORAS, RANK = 8, 16
T_CORE = T_FULL // N_CORES          # 4096 tokens per core
SB_T = 512                          # super-block tokens
N_SB = T_CORE // SB_T               # 8 super-blocks
N_BLK = SB_T // 128                 # 4 token blocks per super-block
KC = D_IN // 128                    # 16 contraction chunks
N_OT = D_OUT // 512                 # 4 o-tiles (full width resident)

_CACHED = {}


def _build(reps=1, lora=True, store=True, xdma=True, shrink1=False):
    # reps>1 wraps the whole body in a device-side For_i loop (same static
    # addresses each iteration) — used only by the timing harness to amortize
    # launch overhead; the graded kernel path uses reps=1.  lora/store/xdma
    # are ablation switches for HW bottleneck attribution (timing harness
    # only — they break correctness).
    key = ("nc", reps, lora, store, xdma, shrink1)
    if key in _CACHED:
        return _CACHED[key]
    kc_eff = KC + 1 if lora else KC  # chunk 16 = fused LoRA expand

    f32 = mybir.dt.float32
    bf16 = mybir.dt.bfloat16

    nc = bacc.Bacc("TRN2", target_bir_lowering=False, debug=False)

    # xP[s, p, c*SB_T + t] = x[s*SB_T + t, c*128 + p]: one contiguous 8 KB run
    # per (super-block, partition) so each super-block is a single 1 MB DMA at
    # line rate (256 B runs of the naive x^T layout pay the <512 B RMW tax).
    xP = nc.dram_tensor("xP", [N_SB * 128, KC * SB_T], bf16, kind="ExternalInput")
    wT = nc.dram_tensor("wT", [D_IN, D_OUT], bf16, kind="ExternalInput")
    aT = nc.dram_tensor("aT", [D_IN, 128], bf16, kind="ExternalInput")
    bT = nc.dram_tensor("bT", [128, D_OUT], bf16, kind="ExternalInput")
    maskM = nc.dram_tensor("maskM", [128, T_CORE], bf16, kind="ExternalInput")
    bias_rep = nc.dram_tensor("bias_rep", [128, D_OUT], f32, kind="ExternalInput")
    out = nc.dram_tensor("out", [T_CORE, D_OUT], f32, kind="ExternalOutput")

    xP_v = xP.rearrange("(s p) q -> p s q", p=128)      # [128, N_SB, KC*SB_T]
    wT_v = wT.rearrange("(c p) o -> p c o", p=128)      # [128, 16, 2048]
    aT_v = aT.rearrange("(c p) r -> p c r", p=128)      # [128, 16, 128]

    with tile.TileContext(nc) as tc:
        with (
            tc.tile_pool(name="const", bufs=1) as const,
            tc.tile_pool(name="wpool", bufs=1) as wpool,
            tc.tile_pool(name="xpool", bufs=3) as xpool,
            tc.tile_pool(name="opool", bufs=3) as opool,
            tc.tile_pool(name="pso", bufs=8, space="PSUM") as pso,
        ):
            at = const.tile([128, KC, 128], bf16)
            bias_t = const.tile([128, D_OUT], bf16)
            mall = const.tile([128, T_CORE], bf16)
            # chunk KC of wt holds the stacked LoRA B table: the expand is
            # just the 17th contraction chunk of the base GEMM.
            wt = wpool.tile([128, KC + 1, D_OUT], bf16)

            rep_cm = tc.For_i(0, reps) if reps > 1 else contextlib.nullcontext()
            with rep_cm:
                def load_x(s, split=1):
                    # chunk KC is filled by shrink()'s DVE mask-multiply.
                    # split>1 loads in c-chunk groups so SB0's prologue
                    # shrink can start after the first group lands.
                    xt = xpool.tile([128, KC + 1, SB_T], bf16, tag="xt",
                                    name=f"xt{s}")
                    gs = KC // split
                    for g in range(split):
                        nc.sync.dma_start(
                            xt[:, g * gs:(g + 1) * gs, :]
                            .rearrange("p c t -> p (c t)"),
                            xP_v[:, s, g * gs * SB_T:(g + 1) * gs * SB_T])
                    return xt

                xts = [None] * N_SB
                # SP-ring order: at (small, gates first shrink LDW), then
                # SB0's x in 2 groups (first shrink MM after group 0 lands);
                # mask/B-table/bias ride SWDGE — not needed for ~10 us.
                nc.sync.dma_start(at[:], aT_v[:])
                xts[0] = load_x(0, split=2)
                for c in range(KC):
                    # W loads split across both HWDGE rings: the SP ring is
                    # mostly idle after SB0's x (2 MB per 57 us of compute),
                    # and ACT alone delivers 8.4 MB slower than block 0
                    # consumes it.  Steady state is unaffected (W resident).
                    eng = nc.sync if c % 2 else nc.scalar
                    eng.dma_start(wt[:, c, :], wT_v[:, c, :])
                nc.gpsimd.dma_start(mall[:], maskM[:])
                nc.gpsimd.dma_start(wt[:, KC, :], bT[:])
                nc.gpsimd.dma_start(bias_t[:], bias_rep[:])  # SWDGE cast

                n_shr = 1 if shrink1 else KC

                def shrink_mm(ps_s, s, xt_s, c):
                    nc.tensor.matmul(ps_s[:, :SB_T], at[:, c, :],
                                     xt_s[:, c, :],
                                     start=(c == 0), stop=(c == n_shr - 1))

                def shrink_fin(ps_s, s, xt_s):
                    # mask zeroes foreign adapters per token column; result
                    # lands in xt chunk KC = the expand's lhsT.
                    nc.vector.tensor_tensor(
                        xt_s[:, KC, :], ps_s[:, :SB_T],
                        mall[:, s * SB_T:(s + 1) * SB_T],
                        mybir.AluOpType.mult)

                def shrink(s, xt_s):
                    # standalone masked LoRA shrink (SB0 prologue only)
                    ps_s = pso.tile([128, 512], f32, tag="ps_o", name="ps_s")
                    for c in range(n_shr):
                        shrink_mm(ps_s, s, xt_s, c)
                    shrink_fin(ps_s, s, xt_s)
                for s in range(N_SB):
                    t0 = s * SB_T
                    if xdma and s + 1 < N_SB:
                        xts[s + 1] = load_x(s + 1)
                    xt = xts[s]
                    if lora and s == 0:
                        # SB0's own shrink, ahead of its block loop: only
                        # block 0's final (c=16) matmul can briefly wait on
                        # the DVE mask-multiply.
                        shrink(0, xt)
                    for b in range(N_BLK):
                        tb = b * 128
                        # next super-block's shrink MMs interleave into block
                        # 1's c-loop (1 shrink MM per 4 base MMs): stationary
                        # reloads hide behind the base group and consecutive
                        # shrink MMs never hit the same PSUM bank
                        # back-to-back — a 16-MM same-bank burst measures
                        # ~2x its roofline cost.
                        inter_shrink = lora and b == 1 and s + 1 < N_SB
                        if inter_shrink:
                            xt_n = xts[s + 1] if xdma else xt
                            ps_s = pso.tile([128, 512], f32, tag="ps_o",
                                            name="ps_s")
                        psums = [
                            pso.tile([128, 512], f32, tag="ps_o",
                                     name=f"ps_o{o}")
                            for o in range(N_OT)
                        ]
                        for c in range(kc_eff):
                            for o in range(N_OT):
                                nc.tensor.matmul(
                                    psums[o][:],
                                    xt[:, c, tb:tb + 128],
                                    wt[:, c, o * 512:(o + 1) * 512],
                                    start=(c == 0),
                                    stop=(c == kc_eff - 1))
                            if inter_shrink and c < n_shr:
                                shrink_mm(ps_s, s + 1, xt_n, c)
                        if inter_shrink:
                            shrink_fin(ps_s, s + 1, xt_n)
                        if not store:
                            continue
                        ot = opool.tile([128, D_OUT], f32, tag="ot", name="ot")
                        last = (s == N_SB - 1 and b == N_BLK - 1)
                        if not last:
                            for o in range(N_OT):
                                nc.vector.tensor_tensor(
                                    ot[:, o * 512:(o + 1) * 512], psums[o][:],
                                    bias_t[:, o * 512:(o + 1) * 512],
                                    mybir.AluOpType.add)
                            # one full-row 1 MB store per block (8 KB/partition
                            # contiguous, line-rate) on the ACT ring, which is
                            # idle once W is loaded — never blocks the x
                            # stream.
                            nc.scalar.dma_start(out[t0 + tb:t0 + tb + 128, :],
                                                ot[:])
                        else:
                            # final block: drain+store in halves so the first
                            # half-store overlaps the second half's drains
                            # (trims the serial tail after the last matmul).
                            half = D_OUT // 2
                            for h in range(2):
                                for o in (2 * h, 2 * h + 1):
                                    nc.vector.tensor_tensor(
                                        ot[:, o * 512:(o + 1) * 512],
                                        psums[o][:],
                                        bias_t[:, o * 512:(o + 1) * 512],
                                        mybir.AluOpType.add)
                                nc.scalar.dma_start(
                                    out[t0 + tb:t0 + tb + 128,
                                        h * half:(h + 1) * half],
                                    ot[:, h * half:(h + 1) * half])

    nc.compile()
    _CACHED[key] = nc
    return nc


def _prep_inputs(x, base_weight, base_bias, lora_a, lora_b, token_lora_indices):
    bf16 = ml_dtypes.bfloat16
    x = np.asarray(x, dtype=np.float32)
    w = np.asarray(base_weight, dtype=np.float32)
    bias = np.asarray(base_bias, dtype=np.float32)
    la = np.asarray(lora_a, dtype=np.float32)
    lb = np.asarray(lora_b, dtype=np.float32)
    idx = np.asarray(token_lora_indices, dtype=np.int32)

    wT = np.ascontiguousarray(w.T).astype(bf16)                      # [D_IN, D_OUT]
    aT = np.ascontiguousarray(la.reshape(128, D_IN).T).astype(bf16)  # [D_IN, 128]
    bT = np.ascontiguousarray(
        lb[:, 0].transpose(0, 2, 1).reshape(128, D_OUT)).astype(bf16)
    bias_rep = np.ascontiguousarray(
        np.broadcast_to(bias[None, :], (128, D_OUT)))                # [128, D_OUT]
    mask = (np.arange(128, dtype=np.int32)[:, None] // RANK
            == idx[None, :]).astype(bf16)                            # [128, T_FULL]

    # xP[s, p, c, t] = x[s*SB_T + t, c*128 + p] per core: one contiguous 8 KB
    # run per (super-block, partition) -> line-rate DMA.
    xP = (x.reshape(N_CORES, N_SB, SB_T, KC, 128)
          .transpose(0, 1, 4, 3, 2)
          .reshape(N_CORES, N_SB * 128, KC * SB_T).astype(bf16))
    in_maps = []
    for c in range(N_CORES):
        sl = slice(c * T_CORE, (c + 1) * T_CORE)
        in_maps.append({
            "xP": np.ascontiguousarray(xP[c]),
            "wT": wT,
            "aT": aT,
            "bT": bT,
            "maskM": np.ascontiguousarray(mask[:, sl]),
            "bias_rep": bias_rep,
        })
    return in_maps


def kernel(x, base_weight, base_bias, lora_a, lora_b, token_lora_indices):
    nc = _build()
    in_maps = _prep_inputs(x, base_weight, base_bias, lora_a, lora_b,
                           token_lora_indices)
    res = run_bass_kernel_spmd(nc, in_maps, list(range(N_CORES)))
    return np.concatenate([res.results[c]["out"] for c in range(N_CORES)], axis=0)

